# revision 3
# baseline (speedup 1.0000x reference)
"""Trainium2 Bass kernel for nn_CrossAttention (packed cross-attention), v3.

Math (verified against the jax reference):
  For each batch b, packed pred rows cross-attend to packed ctx rows:
    Q = Xp_b @ Wq ; [K|V] = Xc_b @ Wkv          (Xp_b, Xc_b: [1024, 512])
    out_b = concat_h( softmax(Q_h K_h^T / 8) V_h ) @ Wproj + bproj
  Softmax needs no max-subtraction: |scores| < ~7, exp is safe in fp32.

Sharding: 8 cores = (2 batches) x (4 head-pairs).  Each core computes two
heads of one batch and the partial output projection for those heads
(row-sharded Wproj); the host sums the 4 partials per batch and adds bproj.

v3 changes over the 54.5us v2 (trace-driven):
  - ALL weights ride one [128, 2048] f16 bundle = 4KB DMA packets (v2's
    1KB weight rows moved at ~82GB/s vs ~290GB/s for 4KB packets), posted
    first on the sync HWDGE queue, then xc, then xp (need-order).
  - warmup matmul ladder sized to the DMA wire time so the first KT
    matmul runs at the full 2.4GHz pstate (v2 idled 2.4us before KT and
    dropped back to the 1.2GHz pstate).
  - softmax exp split across TWO engines: ACT does exact exp on a share
    of (item, query-half) units; the DVE computes the rest with a one-op
    Schraudolph fast-exp -- tensor_scalar affine (s*A+B) into an f32
    carrier whose low halfwords ARE the f16 bits of ~exp(s*scale); the
    PV matmul reads them via a stride-2 bitcast view.  v2 was exp-bound:
    ACT 1.11us/item vs 0.86us of PE work -> the loop ran at 1.37us/item.
  - S psum as 4 rotating [128,512] banks (finer than v2's 2x[128,1024]):
    an S matmul only waits one exp unit 4 slots back.
"""

import sys

if "/opt/trn_rl_repo" not in sys.path:
    sys.path.insert(0, "/opt/trn_rl_repo")

import numpy as np

B, T, N, C, H = 2, 8, 256, 512, 8
T_CTX = T // 2
HD = C // H            # 64
SEQ = T_CTX * N        # 1024 packed tokens per batch (q and kv)
NCORE = 8
CT_N = C // 128        # 4 contraction tiles over C
KT_N = SEQ // 128      # 8 key tiles
QT_N = SEQ // 128      # 8 query tiles
SCALE = HD ** -0.5
SPLIT_WAITS = True  # walrus needs it; CoreSim chokes on it

# fast-exp (Schraudolph, f16-bits-in-f32-carrier):
#   exp(s*SCALE) ~= f16_frombits(low16(f32bits(s*FE_A + FE_B)))
FE_DELTA = 0.045
FE_A = float(SCALE * np.log2(np.e) * 1024.0)
FE_B = float((15.0 - FE_DELTA) * 1024.0 + 12582912.0)

# exp engine assignment per (item, query-half) unit: 32 units total.
# True = DVE fast-exp (approx), False = ACT exact exp.  DVE share trades
# accuracy (~1% output rel err at 50%) for ACT headroom.
DVE_UNIT = [(kt + h + nh) % 2 == 1
            for kt in range(KT_N) for h in range(2) for nh in range(2)]

N_WARM_BIG = 16      # 512-col warmup matmuls (clock ramp during DMA)
N_WARM_SMALL = 6     # 128-col fine-grained tail warmups

_PROG = None


def _build_program():
    import concourse.bass as bass
    import concourse.tile as tile
    from concourse import mybir

    F16 = mybir.dt.float16

    class TrimTailTileContext(tile.TileContext):
        """Skip the second end-of-kernel all-engine barrier: executions of
        the NEFF are serialized by the runtime, and the semaphore clear is
        still ordered after the first barrier on the gpsimd queue."""

        def _drain_and_barrier(self, tick_clock, wait_clock):
            from concourse.vector_clock import ScopedClock

            drain_inst = self.nc.sync.drain()
            wait_clock.add_sem_waits(
                drain_inst.ins, ScopedClock({None: tick_clock.global_clock}))
            self.nc.all_engine_barrier()
            popped = self.nc._tile_sem_poison_stack.pop()
            assert popped is self._sem_poison
            self.nc.clear_and_free_semaphores(
                list(self.sems.allocated().values()))

    nc = bass.Bass("TRN2", target_bir_lowering=False, debug=False,
                   num_devices=NCORE)

    # wAll: [wk | wq | wv | wp] chunk-packed, 4KB rows
    wall = nc.dram_tensor("wall", [128, 4, CT_N, 128], F16,
                          kind="ExternalInput").ap()
    xcP = nc.dram_tensor("xcP", [128, CT_N, SEQ], F16,
                         kind="ExternalInput").ap()
    xpP = nc.dram_tensor("xpP", [128, CT_N, SEQ], F16,
                         kind="ExternalInput").ap()
    out = nc.dram_tensor("out", [SEQ, C], F16, kind="ExternalOutput").ap()

    with TrimTailTileContext(nc) as tc:
        _emit(nc, tc, mybir, wall, xcP, xpP, out)
    if SPLIT_WAITS:
        _split_sync_waits(nc, mybir)
    return nc


def _split_sync_waits(nc, mybir):
    """This container's walrus build has tight per-instruction sync-wait
    limits ("Too many sync wait commands": Matmult holds 1 wait command,
    control-class instructions 2).  Tile freely assigns more.  Rewrite each
    block, moving overflow waits onto same-engine NoOps inserted directly
    before the over-limit instruction (safe: the engine queue executes in
    order, so the waits still complete before the instruction runs)."""
    LIMITS = {}
    DEFAULT = 1
    NOP_W = 1
    n = 0
    for fn in nc.m.functions:
        for bb in fn.blocks:
            insts = bb.instructions
            new = []
            changed = False
            for inst in insts:
                si = inst.sync_info
                waits = list(si.on_wait) if si is not None else []
                limit = LIMITS.get(inst.opcode, DEFAULT)
                if len(waits) > limit:
                    extra = waits[:-limit] if limit else waits
                    keep = waits[-limit:] if limit else []
                    # the end-of-kernel drain carries one wait per logical
                    # processor; spread its nops across engines so they
                    # retire in parallel (the following barrier re-syncs),
                    # instead of ~130ns each serially on the sync sequencer
                    if inst.opcode == "Drain" and len(extra) > 4:
                        engs = [mybir.EngineType.SP, mybir.EngineType.PE,
                                mybir.EngineType.DVE,
                                mybir.EngineType.Activation,
                                mybir.EngineType.Pool]
                    else:
                        engs = [inst.engine]
                    for i in range(0, len(extra), NOP_W):
                        nop = mybir.InstNoOp(
                            name=f"I-waitsplit-{n}", ins=[], outs=[],
                            engine=engs[(i // NOP_W) % len(engs)],
                            sync_info=mybir.SyncInfo(
                                on_wait=extra[i:i + NOP_W], on_update=[]))
                        new.append(nop)
                        n += 1
                    inst.sync_info = mybir.SyncInfo(
                        on_wait=keep, on_update=list(si.on_update))
                    changed = True
                new.append(inst)
            if changed:
                bb.instructions = new


def _emit(nc, tc, mybir, wall, xcP, xpP, out):
    from contextlib import ExitStack

    F32 = mybir.dt.float32
    F16 = mybir.dt.float16
    Exp = mybir.ActivationFunctionType.Exp
    Ln = mybir.ActivationFunctionType.Ln
    Mult = mybir.AluOpType.mult
    Add = mybir.AluOpType.add

    with ExitStack() as ctx:
        sb = ctx.enter_context(tc.tile_pool(name="sb", bufs=1))

        warm = sb.tile([128, 512], F16, tag="warm")
        wall_sb = sb.tile([128, 4, CT_N, 128], F16, tag="wall")
        wk_sb = wall_sb[:, 0]
        wq_sb = wall_sb[:, 1]
        wv_sb = wall_sb[:, 2]
        wp_sb = wall_sb[:, 3].rearrange("p c n -> p (c n)")
        xc_sb = sb.tile([128, CT_N, SEQ], F16, tag="xc")
        xp_sb = sb.tile([128, CT_N, SEQ], F16, tag="xp")
        # per-(head, column-half) tiles: finer dependency granularity lets
        # the first S matmul start after just two (parallel-engine) evacs
        qt_p = [[sb.tile([128, 512], F16, tag=f"qt{h}{nh}",
                         name=f"qt{h}{nh}") for nh in range(2)]
                for h in range(2)]
        kt_p = [[sb.tile([128, 512], F16, tag=f"kt{h}{g}",
                         name=f"kt{h}{g}") for g in range(2)]
                for h in range(2)]
        # wide vones: per kt tile, per head: 64 V cols then 64 ones cols.
        # PV with this lhsT gives psum rows 0-63 = O_h^T, rows 64-127 = Z
        # replicated 64x (so normalization needs no broadcast).
        vones = [sb.tile([128, 4, 256], F16, tag=f"vones{g}", name=f"vones{g}")
                 for g in range(2)]
        # otn per query-half: head0 rows 0-63, head1 rows 64-127 (combined
        # so projection contracts both heads in one K=128 matmul)
        otn = [sb.tile([128, 512], F16, tag=f"otn{nh}", name=f"otn{nh}")
               for nh in range(2)]
        rbc = [sb.tile([64, SEQ], F16, tag=f"rbc{h}", name=f"rbc{h}")
               for h in range(2)]
        zln = [sb.tile([64, SEQ], F16, tag=f"zln{h}", name=f"zln{h}")
               for h in range(2)]
        # exp outputs: ACT units write f16 p tiles; DVE units write f32
        # carriers (low halfwords = f16 bits of the fast-exp)
        pa_t = [sb.tile([128, 512], F16, tag=f"pa{i}", name=f"pa{i}")
                for i in range(3)]
        cb_t = [sb.tile([128, 512], F32, tag=f"cb{i}", name=f"cb{i}")
                for i in range(3)]
        # paired output staging: one contiguous [128, 2, 512] tile per two
        # query tiles -> one strided out-DMA per pair (fewer 600ns issues)
        o16_t = [sb.tile([128, 2, C], F16, tag=f"o16{i}", name=f"o16{i}")
                 for i in range(4)]

        # ---- PE warmup scratch: memset on the (idle) vector engine ----
        nc.vector.memset(warm[:], 0.0)

        # ---- input DMAs: xc then xp need-ordered on the sync HWDGE
        # queue; the weight bundle rides the gpsimd SWDGE queue in
        # parallel.  Every transfer moves >=4KB per partition row ----
        nc.gpsimd.dma_start(out=wall_sb[:], in_=wall)
        nc.sync.dma_start(out=xc_sb[:], in_=xcP)
        nc.sync.dma_start(out=xp_sb[:], in_=xpP)

        # ---- constant / zero-pad memsets (overlap the DMA window) ----
        for g in range(2):
            nc.gpsimd.memset(vones[g][:, :, 64:128], 1.0)
            nc.gpsimd.memset(vones[g][:, :, 192:256], 1.0)
        for g in range(2):
            nc.gpsimd.memset(kt_p[0][g][64:128, :], 0.0)
            nc.gpsimd.memset(kt_p[1][g][0:64, :], 0.0)
            nc.gpsimd.memset(qt_p[0][g][64:128, :], 0.0)
            nc.gpsimd.memset(qt_p[1][g][0:64, :], 0.0)

        # ---- KT then V then QT on the PE (matches data-arrival order) ----
        with ExitStack() as qctx:
            p3_pool = qctx.enter_context(
                tc.tile_pool(name="p3_ps", bufs=1, space="PSUM"))
            # allocation order = psum bank order; banks are recycled into
            # later pools in the same order, so put kt/qt first and warm
            # last (the first S matmul must not wait on late evacuations
            # through bank reuse)
            kt_ps = [p3_pool.tile([128, 512], F32, tag=f"ktps{g}",
                                  name=f"ktps{g}") for g in range(2)]
            qt_ps = [p3_pool.tile([128, 512], F32, tag=f"qtps{nh}",
                                  name=f"qtps{nh}") for nh in range(2)]
            v_ps = [p3_pool.tile([128, 128], F32, tag=f"vps{i}",
                                 name=f"vps{i}") for i in range(2)]
            warm_ps = p3_pool.tile([128, 512], F32, tag="warmps")

            # warmup ladder: ramp the PE clock during the DMA window; the
            # tail is fine-grained so KT starts within ~60ns of data
            for i in range(N_WARM_BIG):
                nc.tensor.matmul(out=warm_ps[:], lhsT=warm[:, 0:128],
                                 rhs=warm[:], start=True, stop=True)
            for i in range(N_WARM_SMALL):
                nc.tensor.matmul(out=warm_ps[:, 0:128], lhsT=warm[:, 0:128],
                                 rhs=warm[:, 0:128], start=True, stop=True)

            # KT: kt_ps[g] = Wk^T Xc for key-half g (accumulate over ct)
            for ct in range(CT_N):
                for g in range(2):
                    nc.tensor.matmul(
                        out=kt_ps[g][:],
                        lhsT=wk_sb[:, ct, :],
                        rhs=xc_sb[:, ct, g * 512:(g + 1) * 512],
                        start=(ct == 0), stop=(ct == CT_N - 1))
            # evacuations: one engine per destination tile (cross-engine
            # writes to one tile serialize through an extra semaphore hop);
            # h=0 on vector, h=1 on scalar so both run concurrently
            for g in range(2):
                nc.vector.tensor_copy(out=kt_p[0][g][0:64, :],
                                      in_=kt_ps[g][0:64, :])
                nc.scalar.copy(out=kt_p[1][g][64:128, :],
                               in_=kt_ps[g][64:128, :])

            for kt in range(KT_N):
                vt = v_ps[kt % 2]
                for ct in range(CT_N):
                    nc.tensor.matmul(
                        out=vt[:],
                        lhsT=xc_sb[:, ct, kt * 128:(kt + 1) * 128],
                        rhs=wv_sb[:, ct, :],
                        start=(ct == 0), stop=(ct == CT_N - 1))
                dst = vones[kt // 4][:, kt % 4, :].rearrange(
                    "p (g s) -> p g s", g=2)[:, :, 0:64]
                vsrc = vt[:].rearrange("p (g s) -> p g s", g=2)
                nc.vector.tensor_copy(out=dst, in_=vsrc)

            for ct in range(CT_N):
                for nh in range(2):
                    nc.tensor.matmul(
                        out=qt_ps[nh][:],
                        lhsT=wq_sb[:, ct, :],
                        rhs=xp_sb[:, ct, nh * 512:(nh + 1) * 512],
                        start=(ct == 0), stop=(ct == CT_N - 1))
            # spread the four qt evacs so item 0's pair (h=0, both query
            # halves) runs on two different engines concurrently
            nc.vector.tensor_copy(out=qt_p[0][0][0:64, :],
                                  in_=qt_ps[0][0:64, :])
            nc.scalar.copy(out=qt_p[0][1][0:64, :],
                           in_=qt_ps[1][0:64, :])
            nc.scalar.copy(out=qt_p[1][0][64:128, :],
                           in_=qt_ps[0][64:128, :])
            nc.vector.tensor_copy(out=qt_p[1][1][64:128, :],
                                  in_=qt_ps[1][64:128, :])

        # ---- attention: S^T -> exp (ACT exact / DVE fast) -> PV ----
        with ExitStack() as actx:
            pv_pool = actx.enter_context(
                tc.tile_pool(name="pv_ps", bufs=1, space="PSUM"))
            pv = [pv_pool.tile([128, SEQ], F32, tag=f"pv{i}", name=f"pv{i}")
                  for i in range(2)]
            s_stack = ExitStack()
            s_pool = s_stack.enter_context(
                tc.tile_pool(name="s_ps", bufs=1, space="PSUM"))
            # 4 rotating S banks, one per (item, query-half) unit; the
            # first pair allocated LAST so it lands on the warm/kt banks
            # (earliest retired)
            s4_late = [s_pool.tile([128, 512], F32, tag=f"s{j}",
                                   name=f"s{j}") for j in (2, 3)]
            s4_early = [s_pool.tile([128, 512], F32, tag=f"s{j}",
                                    name=f"s{j}") for j in (0, 1)]
            s4 = s4_early + s4_late
            items = [(kt, h) for kt in range(KT_N) for h in range(2)]

            # p-slot bookkeeping: rotate independently per engine kind
            na, nb = 0, 0
            unit_src = {}

            def emit_st(i):
                kt, h = items[i]
                for nh in range(2):
                    nc.tensor.matmul(
                        out=s4[(2 * i + nh) % 4][:],
                        lhsT=kt_p[h][kt // 4][:, (kt % 4) * 128:
                                              (kt % 4) * 128 + 128],
                        rhs=qt_p[h][nh][:],
                        start=True, stop=True)

            def emit_exp(i):
                nonlocal na, nb
                for nh in range(2):
                    s = s4[(2 * i + nh) % 4]
                    if DVE_UNIT[2 * i + nh]:
                        c = cb_t[nb % 3]
                        nb += 1
                        nc.vector.tensor_scalar(
                            out=c[:], in0=s[:], scalar1=FE_A, scalar2=FE_B,
                            op0=Mult, op1=Add)
                        unit_src[(i, nh)] = c[:].bitcast(F16).rearrange(
                            "p (c two) -> p c two", two=2)[:, :, 0]
                    else:
                        p = pa_t[na % 3]
                        na += 1
                        nc.scalar.activation(out=p[:], in_=s[:], func=Exp,
                                             scale=float(SCALE))
                        unit_src[(i, nh)] = p[:]

            def emit_pv(i):
                kt, h = items[i]
                for nh in range(2):
                    nc.tensor.matmul(
                        out=pv[h][:, nh * 512:(nh + 1) * 512],
                        lhsT=vones[kt // 4][:, kt % 4,
                                            h * 128:(h + 1) * 128],
                        rhs=unit_src.pop((i, nh)),
                        start=(kt == 0), stop=(kt == KT_N - 1))

            emit_st(0)
            emit_exp(0)
            for i in range(len(items)):
                if i + 1 < len(items):
                    emit_st(i + 1)
                    emit_exp(i + 1)
                emit_pv(i)
            s_stack.close()

            # ---- tail, chunked by query half so projection starts early.
            # 1/Z via ACT ln -> exp(-x) (same act table set as Exp), then
            # otn = O^T/Z and the K=128 projection per query tile.
            with ExitStack() as tctx:
                out_pool = tctx.enter_context(
                    tc.tile_pool(name="out_ps", bufs=1, space="PSUM"))
                out_ps = [out_pool.tile([128, C], F32, tag=f"ops{i}",
                                        name=f"ops{i}") for i in range(4)]
                # PE keepalive through the ln/exp window: an idle PE drops
                # to the 1.2GHz pstate within ~100ns and needs 3us of
                # continuous work to get back to 2.4GHz -- dummy matmuls on
                # the scratch tile bridge the gap until the projections
                for i in range(14):
                    nc.tensor.matmul(out=out_ps[2 + i % 2][:],
                                     lhsT=warm[:, 0:128], rhs=warm[:],
                                     start=True, stop=True)
                # 1/Z = exp(-ln Z) on the replicated Z rows, full width per
                # head (ln and Exp share an act table set: no reload).
                for h in range(2):
                    nc.scalar.activation(out=zln[h][:],
                                         in_=pv[h][64:128, :], func=Ln)
                    nc.scalar.activation(out=rbc[h][:], in_=zln[h][:],
                                         func=Exp, scale=-1.0)
                for nh in range(2):
                    o = nh * 512
                    for h in range(2):
                        nc.vector.tensor_mul(
                            out=otn[nh][h * 64:(h + 1) * 64, :],
                            in0=pv[h][0:64, o:o + 512],
                            in1=rbc[h][:, o:o + 512])
                    for qt in range(nh * 4, nh * 4 + 4):
                        q = (qt % 4) * 128
                        ot = out_ps[qt % 4]
                        nc.tensor.matmul(out=ot[:],
                                         lhsT=otn[nh][:, q:q + 128],
                                         rhs=wp_sb[:], start=True,
                                         stop=True)
                        o16 = o16_t[qt // 2]
                        if qt % 2 == 0:
                            nc.vector.tensor_copy(out=o16[:, 0, :],
                                                  in_=ot[:])
                        else:
                            nc.scalar.copy(out=o16[:, 1, :], in_=ot[:])
                            # alternate out-DMAs across two queues (sync
                            # HWDGE / idle gpsimd SWDGE) so pairs overlap
                            eng = nc.sync if qt % 4 == 1 else nc.gpsimd
                            eng.dma_start(
                                out=out[(qt - 1) * 128:(qt + 1) * 128, :]
                                .rearrange("(k p) c -> p k c", p=128),
                                in_=o16[:])


def _get_program():
    global _PROG
    if _PROG is None:
        _PROG = _build_program()
    return _PROG


def _shard_inputs(x_pred, x_ctx, ctx_mask, Wq, Wkv, Wproj):
    """Build the 8 per-core input maps (host-side sharding + packing)."""
    ctx_mask = np.asarray(ctx_mask).astype(bool)
    pidx = np.nonzero(~ctx_mask.reshape(-1))[0]
    cidx = np.nonzero(ctx_mask.reshape(-1))[0]
    pm = [np.where(pidx // T == b)[0] for b in range(B)]
    cm = [np.where(cidx // T == b)[0] for b in range(B)]
    for b in range(B):
        assert len(pm[b]) == T_CTX and len(cm[b]) == T_CTX, (
            "kernel compiled for T_CTX ctx/pred slots per batch row")

    def pack_x(X):  # [SEQ, C] -> [128, CT_N, SEQ] chunk-packed, 4KB rows
        xt = X.T.astype(np.float16)                 # [C, SEQ]
        return np.ascontiguousarray(
            xt.reshape(CT_N, 128, SEQ).transpose(1, 0, 2))

    def pack_w(W):  # [C, 128] -> [128, CT_N, 128]
        return np.ascontiguousarray(
            W.astype(np.float16).reshape(CT_N, 128, 128).transpose(1, 0, 2))

    xpP_b = [pack_x(x_pred[pm[b]].reshape(SEQ, C)) for b in range(B)]
    xcP_b = [pack_x(x_ctx[cm[b]].reshape(SEQ, C)) for b in range(B)]

    wq16 = Wq.astype(np.float16)
    wk16 = Wkv[:, :C].astype(np.float16)
    wv16 = Wkv[:, C:].astype(np.float16)
    wp16 = Wproj.astype(np.float16)

    in_maps = []
    for c in range(NCORE):
        b, hp = divmod(c, 4)
        hc = hp * 128
        # wall: [wk | wq | wv | wp] -- wp packed as [128, CT_N, 128] by
        # splitting its 512 output cols into 4 chunks
        wall = np.stack([
            pack_w(wk16[:, hc:hc + 128]),
            pack_w(wq16[:, hc:hc + 128]),
            pack_w(wv16[:, hc:hc + 128]),
            np.ascontiguousarray(
                wp16[hc:hc + 128, :].reshape(128, CT_N, 128)),
        ], axis=1)
        in_maps.append({
            "wall": wall,
            "xcP": xcP_b[b],
            "xpP": xpP_b[b],
        })
    return in_maps, pm


def _unshard_output(results, pm, bproj, dtype):
    full = np.zeros((B * T_CTX, N, C), dtype)
    for b in range(B):
        acc = results[4 * b]["out"].astype(np.float64)
        for j in range(1, 4):
            acc = acc + results[4 * b + j]["out"]
        acc = (acc + bproj).astype(dtype)
        full[pm[b]] = acc.reshape(T_CTX, N, C)
    return full


def run(inputs, trace=False, **kwargs):
    """Run the SPMD kernel; returns (full_output, BassKernelResults)."""
    from concourse.bass_utils import run_bass_kernel_spmd

    nc = _get_program()
    in_maps, pm = _shard_inputs(inputs["x_pred"], inputs["x_ctx"],
                                inputs["ctx_mask"], inputs["Wq"],
                                inputs["Wkv"], inputs["Wproj"])
    res = run_bass_kernel_spmd(nc, in_maps, list(range(NCORE)), trace=trace,
                               **kwargs)
    out = _unshard_output(res.results, pm, np.asarray(inputs["bproj"]),
                          np.asarray(inputs["x_pred"]).dtype)
    return out, res


def kernel(x_pred, x_ctx, ctx_mask, Wq, Wkv, Wproj, bproj):
    out, _ = run(dict(x_pred=np.asarray(x_pred), x_ctx=np.asarray(x_ctx),
                      ctx_mask=np.asarray(ctx_mask), Wq=np.asarray(Wq),
                      Wkv=np.asarray(Wkv), Wproj=np.asarray(Wproj),
                      bproj=np.asarray(bproj)))
    return out


# revision 7
# speedup vs baseline: 1.1349x; 1.1349x over previous
"""Trainium2 Bass kernel for nn_CrossAttention (packed cross-attention), v4.

Math (verified against the jax reference):
  For each batch b, packed pred rows cross-attend to packed ctx rows:
    Q = Xp_b @ Wq ; [K|V] = Xc_b @ Wkv          (Xp_b, Xc_b: [1024, 512])
    out_b = concat_h( softmax(Q_h K_h^T / 8) V_h ) @ Wproj + bproj
  Softmax needs no max-subtraction: |scores| < ~7, exp is safe in fp32.

Sharding: 8 cores = (2 batches) x (4 head-pairs).  Each core computes two
heads of one batch and the partial output projection for those heads
(row-sharded Wproj); the host sums the 4 partials per batch and adds bproj.

v4 over the 54.5us v2 (trace-driven):
  - all-weights bundle [128, 4KB rows] FIRST on the sync HWDGE queue,
    then xc and xp each split in half (keys / queries) -- compute starts
    as each piece lands instead of waiting for the full tensor.  All
    packets >= 4KB (v2's 1KB weight rows moved at ~82GB/s vs ~290GB/s).
  - softmax exp split across TWO engines: ACT does exact exp on half the
    (kt, h, nh) units; the DVE computes the rest with a one-op
    Schraudolph fast-exp -- tensor_scalar affine (s*A+B) into an f32
    carrier whose low halfwords ARE the f16 bits of ~exp(s*scale); the
    PV matmul reads them through a stride-2 bitcast view.  v2 was
    exp-bound (ACT 1.11us/item vs 0.86us of PE work).
  - query-half-major loop: all nh=0 units first, so the nh=0 softmax
    normalization, projection and output DMA overlap the nh=1 half of
    the loop; only the nh=1 tail remains after the last PV.
  - output written as [pair, 128, 2, 512] blocks (2KB DMA rows); the
    host un-interleaves (v2's row-strided stores moved 1KB packets).
"""

import sys

if "/opt/trn_rl_repo" not in sys.path:
    sys.path.insert(0, "/opt/trn_rl_repo")

import numpy as np

B, T, N, C, H = 2, 8, 256, 512, 8
T_CTX = T // 2
HD = C // H            # 64
SEQ = T_CTX * N        # 1024 packed tokens per batch (q and kv)
NCORE = 8
CT_N = C // 128        # 4 contraction tiles over C
KT_N = SEQ // 128      # 8 key tiles
SCALE = HD ** -0.5
SPLIT_WAITS = True  # walrus needs it; CoreSim chokes on it

# fast-exp (Schraudolph, f16-bits-in-f32-carrier):
#   exp(s*SCALE) ~= f16_frombits(low16(f32bits(s*FE_A + FE_B)))
FE_DELTA = 0.045
FE_A = float(SCALE * np.log2(np.e) * 1024.0)
FE_B = float((15.0 - FE_DELTA) * 1024.0 + 12582912.0)

# exp engine per (nh, kt, h) unit: True = DVE fast-exp (approx), False =
# ACT exact exp.  50% DVE -> ~1.0e-2 output rel err (gate is 2e-2).
DVE_UNIT = [g % 2 == 1 for g in range(2 * KT_N * 2)]

N_WARM_BIG = 10      # 512-col warmup matmuls (clock ramp during DMA)
N_WARM_SMALL = 6     # 128-col fine-grained tail warmups

_PROG = None


def _build_program():
    import concourse.bass as bass
    import concourse.tile as tile
    from concourse import mybir

    F16 = mybir.dt.float16

    class TrimTailTileContext(tile.TileContext):
        """Skip the second end-of-kernel all-engine barrier: executions of
        the NEFF are serialized by the runtime, and the semaphore clear is
        still ordered after the first barrier on the gpsimd queue."""

        def _drain_and_barrier(self, tick_clock, wait_clock):
            from concourse.vector_clock import ScopedClock

            drain_inst = self.nc.sync.drain()
            wait_clock.add_sem_waits(
                drain_inst.ins, ScopedClock({None: tick_clock.global_clock}))
            self.nc.all_engine_barrier()
            popped = self.nc._tile_sem_poison_stack.pop()
            assert popped is self._sem_poison
            self.nc.clear_and_free_semaphores(
                list(self.sems.allocated().values()))

    nc = bass.Bass("TRN2", target_bir_lowering=False, debug=False,
                   num_devices=NCORE)

    # wAll: [wk | wq | wv | wp] chunk-packed, 4KB rows
    wall = nc.dram_tensor("wall", [128, 4, CT_N, 128], F16,
                          kind="ExternalInput").ap()
    xcA = nc.dram_tensor("xcA", [128, CT_N, 512], F16,
                         kind="ExternalInput").ap()
    xcB = nc.dram_tensor("xcB", [128, CT_N, 512], F16,
                         kind="ExternalInput").ap()
    xpA = nc.dram_tensor("xpA", [128, CT_N, 512], F16,
                         kind="ExternalInput").ap()
    xpB = nc.dram_tensor("xpB", [128, CT_N, 512], F16,
                         kind="ExternalInput").ap()
    # out as 4 pair-blocks [128, 2, 512] (2KB rows); host un-interleaves
    out = nc.dram_tensor("out", [4, 128, 2, C], F16,
                         kind="ExternalOutput").ap()

    with TrimTailTileContext(nc) as tc:
        _emit(nc, tc, mybir, wall, [xcA, xcB], [xpA, xpB], out)
    if SPLIT_WAITS:
        _split_sync_waits(nc, mybir)
    return nc


def _split_sync_waits(nc, mybir):
    """This container's walrus build has tight per-instruction sync-wait
    limits ("Too many sync wait commands": Matmult holds 1 wait command,
    control-class instructions 2).  Tile freely assigns more.  Rewrite each
    block, moving overflow waits onto same-engine NoOps inserted directly
    before the over-limit instruction (safe: the engine queue executes in
    order, so the waits still complete before the instruction runs)."""
    LIMITS = {}
    DEFAULT = 1
    NOP_W = 1
    n = 0
    for fn in nc.m.functions:
        for bb in fn.blocks:
            insts = bb.instructions
            new = []
            changed = False
            for inst in insts:
                si = inst.sync_info
                waits = list(si.on_wait) if si is not None else []
                limit = LIMITS.get(inst.opcode, DEFAULT)
                if len(waits) > limit:
                    extra = waits[:-limit] if limit else waits
                    keep = waits[-limit:] if limit else []
                    # the end-of-kernel drain carries one wait per logical
                    # processor; spread its nops across engines so they
                    # retire in parallel (the following barrier re-syncs),
                    # instead of ~130ns each serially on the sync sequencer
                    if inst.opcode == "Drain" and len(extra) > 4:
                        engs = [mybir.EngineType.SP, mybir.EngineType.PE,
                                mybir.EngineType.DVE,
                                mybir.EngineType.Activation,
                                mybir.EngineType.Pool]
                    else:
                        engs = [inst.engine]
                    for i in range(0, len(extra), NOP_W):
                        nop = mybir.InstNoOp(
                            name=f"I-waitsplit-{n}", ins=[], outs=[],
                            engine=engs[(i // NOP_W) % len(engs)],
                            sync_info=mybir.SyncInfo(
                                on_wait=extra[i:i + NOP_W], on_update=[]))
                        new.append(nop)
                        n += 1
                    inst.sync_info = mybir.SyncInfo(
                        on_wait=keep, on_update=list(si.on_update))
                    changed = True
                new.append(inst)
            if changed:
                bb.instructions = new


def _emit(nc, tc, mybir, wall, xc_d, xp_d, out):
    from contextlib import ExitStack

    F32 = mybir.dt.float32
    F16 = mybir.dt.float16
    Exp = mybir.ActivationFunctionType.Exp
    Ln = mybir.ActivationFunctionType.Ln
    Mult = mybir.AluOpType.mult
    Add = mybir.AluOpType.add

    with ExitStack() as ctx:
        sb = ctx.enter_context(tc.tile_pool(name="sb", bufs=1))

        warm = sb.tile([128, 512], F16, tag="warm")
        wall_sb = sb.tile([128, 4, CT_N, 128], F16, tag="wall")
        wk_sb = wall_sb[:, 0]
        wq_sb = wall_sb[:, 1]
        wv_sb = wall_sb[:, 2]
        wp_sb = wall_sb[:, 3].rearrange("p c n -> p (c n)")
        xc_sb = [sb.tile([128, CT_N, 512], F16, tag=f"xc{g}", name=f"xc{g}")
                 for g in range(2)]
        xp_sb = [sb.tile([128, CT_N, 512], F16, tag=f"xp{nh}",
                         name=f"xp{nh}") for nh in range(2)]
        qt_p = [[sb.tile([128, 512], F16, tag=f"qt{h}{nh}",
                         name=f"qt{h}{nh}") for nh in range(2)]
                for h in range(2)]
        kt_p = [[sb.tile([128, 512], F16, tag=f"kt{h}{g}",
                         name=f"kt{h}{g}") for g in range(2)]
                for h in range(2)]
        # wide vones: per kt tile, per head: 64 V cols then 64 ones cols.
        # PV with this lhsT gives psum rows 0-63 = O_h^T, rows 64-127 = Z
        # replicated 64x (so normalization needs no broadcast).
        vones = [sb.tile([128, 4, 256], F16, tag=f"vones{g}", name=f"vones{g}")
                 for g in range(2)]
        otn = [sb.tile([128, 512], F16, tag=f"otn{nh}", name=f"otn{nh}")
               for nh in range(2)]
        rbc = [[sb.tile([64, 512], F16, tag=f"rbc{nh}{h}",
                        name=f"rbc{nh}{h}") for h in range(2)]
               for nh in range(2)]
        zln = [[sb.tile([64, 512], F16, tag=f"zln{nh}{h}",
                        name=f"zln{nh}{h}") for h in range(2)]
               for nh in range(2)]
        # exp outputs: ACT units write f16 p tiles; DVE units write f32
        # carriers (low halfwords = f16 bits of the fast-exp)
        pa_t = [sb.tile([128, 512], F16, tag=f"pa{i}", name=f"pa{i}")
                for i in range(3)]
        cb_t = [sb.tile([128, 512], F32, tag=f"cb{i}", name=f"cb{i}")
                for i in range(3)]
        # output staging: one contiguous [128, 2, 512] tile per qt pair
        o16_t = [sb.tile([128, 2, C], F16, tag=f"o16{i}", name=f"o16{i}")
                 for i in range(4)]

        # ---- PE warmup scratch: memset on the (idle) vector engine ----
        nc.vector.memset(warm[:], 0.0)

        # ---- input DMAs, one need-ordered sync HWDGE queue: weights
        # first (everything contracts against them), then xc halves
        # (keys), then xp halves (queries).  4-8KB packets throughout ----
        nc.sync.dma_start(out=wall_sb[:], in_=wall)
        nc.sync.dma_start(out=xc_sb[0][:], in_=xc_d[0])
        nc.sync.dma_start(out=xc_sb[1][:], in_=xc_d[1])
        nc.sync.dma_start(out=xp_sb[0][:], in_=xp_d[0])
        nc.sync.dma_start(out=xp_sb[1][:], in_=xp_d[1])

        # ---- constant / zero-pad memsets (overlap the DMA window) ----
        for g in range(2):
            nc.gpsimd.memset(vones[g][:, :, 64:128], 1.0)
            nc.gpsimd.memset(vones[g][:, :, 192:256], 1.0)
        for g in range(2):
            nc.gpsimd.memset(kt_p[0][g][64:128, :], 0.0)
            nc.gpsimd.memset(kt_p[1][g][0:64, :], 0.0)
            nc.gpsimd.memset(qt_p[0][g][64:128, :], 0.0)
            nc.gpsimd.memset(qt_p[1][g][0:64, :], 0.0)

        # ---- psum pools with hand-managed lifetimes (8 banks total).
        # Two independent LIFO sides: left holds {kq} -> {pv1} -> {proj},
        # right holds {warm} -> {v} -> {s3, pv0}; peak usage is exactly 8.
        p3_stack = ExitStack()
        p3 = p3_stack.enter_context(
            tc.tile_pool(name="p3_ps", bufs=1, space="PSUM", side="left"))
        kq = [p3.tile([128, 512], F32, tag=f"kq{g}", name=f"kq{g}")
              for g in range(2)]
        warm_stack = ExitStack()
        wpool = warm_stack.enter_context(
            tc.tile_pool(name="warm_ps", bufs=1, space="PSUM", side="right"))
        warm_ps = wpool.tile([128, 512], F32, tag="warmps")

        # warmup ladder: ramp the PE clock during the DMA window; the
        # tail is fine-grained so KT starts within ~60ns of data arrival
        for i in range(N_WARM_BIG):
            nc.tensor.matmul(out=warm_ps[:], lhsT=warm[:, 0:128],
                             rhs=warm[:], start=True, stop=True)
        for i in range(N_WARM_SMALL):
            nc.tensor.matmul(out=warm_ps[:, 0:128], lhsT=warm[:, 0:128],
                             rhs=warm[:, 0:128], start=True, stop=True)
        warm_stack.close()
        v_stack = ExitStack()
        vpool = v_stack.enter_context(
            tc.tile_pool(name="v_ps", bufs=1, space="PSUM", side="right"))
        v_ps = vpool.tile([128, 128], F32, tag="vps")

        def emit_ktv(g):
            # KT: kq[g] = Wk^T Xc_g (accumulate over ct), then evac with
            # one engine per destination tile; V for this key-half
            for ct in range(CT_N):
                nc.tensor.matmul(
                    out=kq[g][:], lhsT=wk_sb[:, ct, :],
                    rhs=xc_sb[g][:, ct, :],
                    start=(ct == 0), stop=(ct == CT_N - 1))
            nc.vector.tensor_copy(out=kt_p[0][g][0:64, :],
                                  in_=kq[g][0:64, :])
            nc.scalar.copy(out=kt_p[1][g][64:128, :],
                           in_=kq[g][64:128, :])
            for kt in range(4 * g, 4 * g + 4):
                for ct in range(CT_N):
                    nc.tensor.matmul(
                        out=v_ps[:],
                        lhsT=xc_sb[g][:, ct, (kt % 4) * 128:
                                      (kt % 4) * 128 + 128],
                        rhs=wv_sb[:, ct, :],
                        start=(ct == 0), stop=(ct == CT_N - 1))
                dst = vones[g][:, kt % 4, :].rearrange(
                    "p (q s) -> p q s", q=2)[:, :, 0:64]
                vsrc = v_ps[:].rearrange("p (q s) -> p q s", q=2)
                nc.vector.tensor_copy(out=dst, in_=vsrc)

        def emit_qt(nh):
            # QT for query-half nh into kq[nh] (banks free after KT evac)
            for ct in range(CT_N):
                nc.tensor.matmul(
                    out=kq[nh][:], lhsT=wq_sb[:, ct, :],
                    rhs=xp_sb[nh][:, ct, :],
                    start=(ct == 0), stop=(ct == CT_N - 1))
            if nh == 0:
                nc.vector.tensor_copy(out=qt_p[0][nh][0:64, :],
                                      in_=kq[nh][0:64, :])
                nc.scalar.copy(out=qt_p[1][nh][64:128, :],
                               in_=kq[nh][64:128, :])
            else:
                nc.scalar.copy(out=qt_p[0][nh][0:64, :],
                               in_=kq[nh][0:64, :])
                nc.vector.tensor_copy(out=qt_p[1][nh][64:128, :],
                                      in_=kq[nh][64:128, :])

        emit_ktv(0)
        emit_ktv(1)
        v_stack.close()
        emit_qt(0)

        # ---- attention loop: query-half-major; per unit one S matmul,
        # one exp (ACT exact / DVE fast alternating), one PV matmul ----
        s_stack = ExitStack()
        s_pool = s_stack.enter_context(
            tc.tile_pool(name="s_ps", bufs=1, space="PSUM", side="right"))
        s3 = [s_pool.tile([128, 512], F32, tag=f"s{j}", name=f"s{j}")
              for j in range(3)]
        pv0_stack = ExitStack()
        pv0_pool = pv0_stack.enter_context(
            tc.tile_pool(name="pv0_ps", bufs=1, space="PSUM", side="right"))
        pv_n = [[pv0_pool.tile([128, 512], F32, tag=f"pv0{h}",
                               name=f"pv0{h}") for h in range(2)], None]

        units = [(kt, h) for kt in range(KT_N) for h in range(2)]
        na, nb = 0, 0
        unit_src = {}

        def emit_st(nh, u):
            kt, h = units[u]
            nc.tensor.matmul(
                out=s3[u % 3][:],
                lhsT=kt_p[h][kt // 4][:, (kt % 4) * 128:(kt % 4) * 128 + 128],
                rhs=qt_p[h][nh][:], start=True, stop=True)

        def emit_exp(nh, u):
            nonlocal na, nb
            s = s3[u % 3]
            if DVE_UNIT[nh * 16 + u]:
                c = cb_t[nb % 3]
                nb += 1
                nc.vector.tensor_scalar(
                    out=c[:], in0=s[:], scalar1=FE_A, scalar2=FE_B,
                    op0=Mult, op1=Add)
                unit_src[u] = c[:].bitcast(F16).rearrange(
                    "p (c two) -> p c two", two=2)[:, :, 0]
            else:
                p = pa_t[na % 3]
                na += 1
                nc.scalar.activation(out=p[:], in_=s[:], func=Exp,
                                     scale=float(SCALE))
                unit_src[u] = p[:]

        def emit_pv(nh, u):
            kt, h = units[u]
            nc.tensor.matmul(
                out=pv_n[nh][h][:],
                lhsT=vones[kt // 4][:, kt % 4, h * 128:(h + 1) * 128],
                rhs=unit_src.pop(u),
                start=(kt == 0), stop=(kt == KT_N - 1))

        def emit_rbc(nh, h):
            # 1/Z = exp(-ln Z) on the replicated Z rows (ln and Exp share
            # an act table set: no reload)
            nc.scalar.activation(out=zln[nh][h][:],
                                 in_=pv_n[nh][h][64:128, :], func=Ln)
            nc.scalar.activation(out=rbc[nh][h][:], in_=zln[nh][h][:],
                                 func=Exp, scale=-1.0)

        def emit_otn(nh, h):
            nc.vector.tensor_mul(out=otn[nh][h * 64:(h + 1) * 64, :],
                                 in0=pv_n[nh][h][0:64, :],
                                 in1=rbc[nh][h][:])

        def emit_proj(nh, qts, out_ps):
            # projection per 128-query tile + staged evac; DMA per pair
            for j, q in enumerate(qts):
                ot = out_ps[j % len(out_ps)]
                nc.tensor.matmul(out=ot[:],
                                 lhsT=otn[nh][:, q * 128:q * 128 + 128],
                                 rhs=wp_sb[:], start=True, stop=True)
                qt = nh * 4 + q
                o16 = o16_t[qt // 2]
                if qt % 2 == 0:
                    nc.vector.tensor_copy(out=o16[:, 0, :], in_=ot[:])
                else:
                    nc.scalar.copy(out=o16[:, 1, :], in_=ot[:])
                    nc.sync.dma_start(out=out[qt // 2], in_=o16[:])

        # --- nh=0 group ---
        for u in range(3):
            emit_st(0, u)
            emit_exp(0, u)
        for u in range(len(units)):
            if u == 2:
                # QT-B slots in once xpB lands; kq[1] free after KT evac
                emit_qt(1)
                p3_stack.close()
            if u + 3 < len(units):
                emit_st(0, u + 3)
                emit_exp(0, u + 3)
            emit_pv(0, u)

        pv1_stack = ExitStack()
        pv1_pool = pv1_stack.enter_context(
            tc.tile_pool(name="pv1_ps", bufs=1, space="PSUM", side="left"))
        pv_n[1] = [pv1_pool.tile([128, 512], F32, tag=f"pv1{h}",
                                 name=f"pv1{h}") for h in range(2)]

        # --- nh=1 group, with the nh=0 normalize/project interleaved ---
        for u in range(3):
            emit_st(1, u)
            emit_exp(1, u)
        proj_stack = ExitStack()
        out_ps = None
        for u in range(len(units)):
            if u == 2:
                emit_rbc(0, 0)
            if u == 4:
                emit_rbc(0, 1)
            if u == 6:
                emit_otn(0, 0)
                emit_otn(0, 1)
            if u == 8:
                # pv0 banks free after the otn reads -> projection psum
                pv0_stack.close()
                opool = proj_stack.enter_context(
                    tc.tile_pool(name="proj_ps", bufs=1, space="PSUM",
                                 side="left"))
                out_ps = [opool.tile([128, C], F32, tag=f"ops{i}",
                                     name=f"ops{i}") for i in range(3)]
            if u == 9:
                emit_proj(0, [0, 1], out_ps)
            if u == 11:
                emit_proj(0, [2, 3], out_ps)
            if u + 3 < len(units):
                emit_st(1, u + 3)
                emit_exp(1, u + 3)
            emit_pv(1, u)

        # --- nh=1 tail: keepalives bridge the rbc window, then the
        # normalize -> project -> evac -> DMA chain, finest last ---
        for i in range(8):
            nc.tensor.matmul(out=out_ps[2][:], lhsT=warm[:, 0:128],
                             rhs=warm[:], start=True, stop=True)
        emit_rbc(1, 0)
        emit_rbc(1, 1)
        emit_otn(1, 0)
        emit_otn(1, 1)
        emit_proj(1, [0, 1], out_ps)
        emit_proj(1, [2, 3], out_ps)
        proj_stack.close()
        pv1_stack.close()
        s_stack.close()


def _get_program():
    global _PROG
    if _PROG is None:
        _PROG = _build_program()
    return _PROG


def _shard_inputs(x_pred, x_ctx, ctx_mask, Wq, Wkv, Wproj):
    """Build the 8 per-core input maps (host-side sharding + packing)."""
    ctx_mask = np.asarray(ctx_mask).astype(bool)
    pidx = np.nonzero(~ctx_mask.reshape(-1))[0]
    cidx = np.nonzero(ctx_mask.reshape(-1))[0]
    pm = [np.where(pidx // T == b)[0] for b in range(B)]
    cm = [np.where(cidx // T == b)[0] for b in range(B)]
    for b in range(B):
        assert len(pm[b]) == T_CTX and len(cm[b]) == T_CTX, (
            "kernel compiled for T_CTX ctx/pred slots per batch row")

    def pack_x(X):  # [SEQ, C] -> two [128, CT_N, 512] halves (4KB rows)
        xt = X.T.astype(np.float16)                 # [C, SEQ]
        full = xt.reshape(CT_N, 128, SEQ).transpose(1, 0, 2)
        return (np.ascontiguousarray(full[:, :, :512]),
                np.ascontiguousarray(full[:, :, 512:]))

    def pack_w(W):  # [C, 128] -> [128, CT_N, 128]
        return np.ascontiguousarray(
            W.astype(np.float16).reshape(CT_N, 128, 128).transpose(1, 0, 2))

    xp_b = [pack_x(x_pred[pm[b]].reshape(SEQ, C)) for b in range(B)]
    xc_b = [pack_x(x_ctx[cm[b]].reshape(SEQ, C)) for b in range(B)]

    wq16 = Wq.astype(np.float16)
    wk16 = Wkv[:, :C].astype(np.float16)
    wv16 = Wkv[:, C:].astype(np.float16)
    wp16 = Wproj.astype(np.float16)

    in_maps = []
    for c in range(NCORE):
        b, hp = divmod(c, 4)
        hc = hp * 128
        wall = np.stack([
            pack_w(wk16[:, hc:hc + 128]),
            pack_w(wq16[:, hc:hc + 128]),
            pack_w(wv16[:, hc:hc + 128]),
            np.ascontiguousarray(
                wp16[hc:hc + 128, :].reshape(128, CT_N, 128)),
        ], axis=1)
        in_maps.append({
            "wall": wall,
            "xcA": xc_b[b][0], "xcB": xc_b[b][1],
            "xpA": xp_b[b][0], "xpB": xp_b[b][1],
        })
    return in_maps, pm


def _unshard_output(results, pm, bproj, dtype):
    full = np.zeros((B * T_CTX, N, C), dtype)
    for b in range(B):
        # out blocks [4, 128, 2, 512]: row q = j*256 + k*128 + p
        acc = results[4 * b]["out"].astype(np.float64)
        for j in range(1, 4):
            acc = acc + results[4 * b + j]["out"]
        acc = acc.transpose(0, 2, 1, 3).reshape(SEQ, C)
        acc = (acc + bproj).astype(dtype)
        full[pm[b]] = acc.reshape(T_CTX, N, C)
    return full


def run(inputs, trace=False, **kwargs):
    """Run the SPMD kernel; returns (full_output, BassKernelResults)."""
    from concourse.bass_utils import run_bass_kernel_spmd

    nc = _get_program()
    in_maps, pm = _shard_inputs(inputs["x_pred"], inputs["x_ctx"],
                                inputs["ctx_mask"], inputs["Wq"],
                                inputs["Wkv"], inputs["Wproj"])
    res = run_bass_kernel_spmd(nc, in_maps, list(range(NCORE)), trace=trace,
                               **kwargs)
    out = _unshard_output(res.results, pm, np.asarray(inputs["bproj"]),
                          np.asarray(inputs["x_pred"]).dtype)
    return out, res


def kernel(x_pred, x_ctx, ctx_mask, Wq, Wkv, Wproj, bproj):
    out, _ = run(dict(x_pred=np.asarray(x_pred), x_ctx=np.asarray(x_ctx),
                      ctx_mask=np.asarray(ctx_mask), Wq=np.asarray(Wq),
                      Wkv=np.asarray(Wkv), Wproj=np.asarray(Wproj),
                      bproj=np.asarray(bproj)))
    return out


# revision 8
# speedup vs baseline: 1.1556x; 1.0182x over previous
"""Trainium2 Bass kernel for nn_CrossAttention (packed cross-attention), v4.

Math (verified against the jax reference):
  For each batch b, packed pred rows cross-attend to packed ctx rows:
    Q = Xp_b @ Wq ; [K|V] = Xc_b @ Wkv          (Xp_b, Xc_b: [1024, 512])
    out_b = concat_h( softmax(Q_h K_h^T / 8) V_h ) @ Wproj + bproj
  Softmax needs no max-subtraction: |scores| < ~7, exp is safe in fp32.

Sharding: 8 cores = (2 batches) x (4 head-pairs).  Each core computes two
heads of one batch and the partial output projection for those heads
(row-sharded Wproj); the host sums the 4 partials per batch and adds bproj.

v4 over the 54.5us v2 (trace-driven):
  - all-weights bundle [128, 4KB rows] FIRST on the sync HWDGE queue,
    then xc and xp each split in half (keys / queries) -- compute starts
    as each piece lands instead of waiting for the full tensor.  All
    packets >= 4KB (v2's 1KB weight rows moved at ~82GB/s vs ~290GB/s).
  - softmax exp split across TWO engines: ACT does exact exp on half the
    (kt, h, nh) units; the DVE computes the rest with a one-op
    Schraudolph fast-exp -- tensor_scalar affine (s*A+B) into an f32
    carrier whose low halfwords ARE the f16 bits of ~exp(s*scale); the
    PV matmul reads them through a stride-2 bitcast view.  v2 was
    exp-bound (ACT 1.11us/item vs 0.86us of PE work).
  - query-half-major loop: all nh=0 units first, so the nh=0 softmax
    normalization, projection and output DMA overlap the nh=1 half of
    the loop; only the nh=1 tail remains after the last PV.
  - output written as [pair, 128, 2, 512] blocks (2KB DMA rows); the
    host un-interleaves (v2's row-strided stores moved 1KB packets).
"""

import sys

if "/opt/trn_rl_repo" not in sys.path:
    sys.path.insert(0, "/opt/trn_rl_repo")

import numpy as np

B, T, N, C, H = 2, 8, 256, 512, 8
T_CTX = T // 2
HD = C // H            # 64
SEQ = T_CTX * N        # 1024 packed tokens per batch (q and kv)
NCORE = 8
CT_N = C // 128        # 4 contraction tiles over C
KT_N = SEQ // 128      # 8 key tiles
SCALE = HD ** -0.5
SPLIT_WAITS = True  # walrus needs it; CoreSim chokes on it

# fast-exp (Schraudolph, f16-bits-in-f32-carrier):
#   exp(s*SCALE) ~= f16_frombits(low16(f32bits(s*FE_A + FE_B)))
FE_DELTA = 0.045
FE_A = float(SCALE * np.log2(np.e) * 1024.0)
FE_B = float((15.0 - FE_DELTA) * 1024.0 + 12582912.0)

# exp engine per (nh, kt, h) unit: True = DVE fast-exp (approx), False =
# ACT exact exp.  50% DVE -> ~1.0e-2 output rel err (gate is 2e-2).
DVE_UNIT = [g % 2 == 1 for g in range(KT_N * 2)] + [
    False, True, False, True, False, True, False, True,
    False, True, False, True, False, False, True, True]

N_WARM_BIG = 10      # 512-col warmup matmuls (clock ramp during DMA)
N_WARM_SMALL = 6     # 128-col fine-grained tail warmups

_PROG = None


def _build_program():
    import concourse.bass as bass
    import concourse.tile as tile
    from concourse import mybir

    F16 = mybir.dt.float16

    class TrimTailTileContext(tile.TileContext):
        """Skip the second end-of-kernel all-engine barrier: executions of
        the NEFF are serialized by the runtime, and the semaphore clear is
        still ordered after the first barrier on the gpsimd queue."""

        def _drain_and_barrier(self, tick_clock, wait_clock):
            from concourse.vector_clock import ScopedClock

            drain_inst = self.nc.sync.drain()
            wait_clock.add_sem_waits(
                drain_inst.ins, ScopedClock({None: tick_clock.global_clock}))
            self.nc.all_engine_barrier()
            popped = self.nc._tile_sem_poison_stack.pop()
            assert popped is self._sem_poison
            self.nc.clear_and_free_semaphores(
                list(self.sems.allocated().values()))

    nc = bass.Bass("TRN2", target_bir_lowering=False, debug=False,
                   num_devices=NCORE)

    # wAll: [wk | wq | wv | wp] chunk-packed, 4KB rows
    wall = nc.dram_tensor("wall", [128, 4, CT_N, 128], F16,
                          kind="ExternalInput").ap()
    xcA = nc.dram_tensor("xcA", [128, CT_N, 512], F16,
                         kind="ExternalInput").ap()
    xcB = nc.dram_tensor("xcB", [128, CT_N, 512], F16,
                         kind="ExternalInput").ap()
    xpA = nc.dram_tensor("xpA", [128, CT_N, 512], F16,
                         kind="ExternalInput").ap()
    xpB = nc.dram_tensor("xpB", [128, CT_N, 512], F16,
                         kind="ExternalInput").ap()
    # out as 4 pair-blocks [128, 2, 512] (2KB rows); host un-interleaves
    out = nc.dram_tensor("out", [4, 128, 2, C], F16,
                         kind="ExternalOutput").ap()

    with TrimTailTileContext(nc) as tc:
        _emit(nc, tc, mybir, wall, [xcA, xcB], [xpA, xpB], out)
    if SPLIT_WAITS:
        _split_sync_waits(nc, mybir)
    return nc


def _split_sync_waits(nc, mybir):
    """This container's walrus build has tight per-instruction sync-wait
    limits ("Too many sync wait commands": Matmult holds 1 wait command,
    control-class instructions 2).  Tile freely assigns more.  Rewrite each
    block, moving overflow waits onto same-engine NoOps inserted directly
    before the over-limit instruction (safe: the engine queue executes in
    order, so the waits still complete before the instruction runs)."""
    LIMITS = {}
    DEFAULT = 1
    NOP_W = 1
    n = 0
    for fn in nc.m.functions:
        for bb in fn.blocks:
            insts = bb.instructions
            new = []
            changed = False
            for inst in insts:
                si = inst.sync_info
                waits = list(si.on_wait) if si is not None else []
                limit = LIMITS.get(inst.opcode, DEFAULT)
                if len(waits) > limit:
                    extra = waits[:-limit] if limit else waits
                    keep = waits[-limit:] if limit else []
                    # the end-of-kernel drain carries one wait per logical
                    # processor; spread its nops across engines so they
                    # retire in parallel (the following barrier re-syncs),
                    # instead of ~130ns each serially on the sync sequencer
                    if inst.opcode == "Drain" and len(extra) > 4:
                        engs = [mybir.EngineType.SP, mybir.EngineType.PE,
                                mybir.EngineType.DVE,
                                mybir.EngineType.Activation,
                                mybir.EngineType.Pool]
                    else:
                        engs = [inst.engine]
                    for i in range(0, len(extra), NOP_W):
                        nop = mybir.InstNoOp(
                            name=f"I-waitsplit-{n}", ins=[], outs=[],
                            engine=engs[(i // NOP_W) % len(engs)],
                            sync_info=mybir.SyncInfo(
                                on_wait=extra[i:i + NOP_W], on_update=[]))
                        new.append(nop)
                        n += 1
                    inst.sync_info = mybir.SyncInfo(
                        on_wait=keep, on_update=list(si.on_update))
                    changed = True
                new.append(inst)
            if changed:
                bb.instructions = new


def _emit(nc, tc, mybir, wall, xc_d, xp_d, out):
    from contextlib import ExitStack

    F32 = mybir.dt.float32
    F16 = mybir.dt.float16
    Exp = mybir.ActivationFunctionType.Exp
    Ln = mybir.ActivationFunctionType.Ln
    Mult = mybir.AluOpType.mult
    Add = mybir.AluOpType.add

    with ExitStack() as ctx:
        sb = ctx.enter_context(tc.tile_pool(name="sb", bufs=1))

        warm = sb.tile([128, 512], F16, tag="warm")
        wall_sb = sb.tile([128, 4, CT_N, 128], F16, tag="wall")
        wk_sb = wall_sb[:, 0]
        wq_sb = wall_sb[:, 1]
        wv_sb = wall_sb[:, 2]
        wp_sb = wall_sb[:, 3].rearrange("p c n -> p (c n)")
        xc_sb = [sb.tile([128, CT_N, 512], F16, tag=f"xc{g}", name=f"xc{g}")
                 for g in range(2)]
        xp_sb = [sb.tile([128, CT_N, 512], F16, tag=f"xp{nh}",
                         name=f"xp{nh}") for nh in range(2)]
        qt_p = [[sb.tile([128, 512], F16, tag=f"qt{h}{nh}",
                         name=f"qt{h}{nh}") for nh in range(2)]
                for h in range(2)]
        kt_p = [[sb.tile([128, 512], F16, tag=f"kt{h}{g}",
                         name=f"kt{h}{g}") for g in range(2)]
                for h in range(2)]
        # wide vones: per kt tile, per head: 64 V cols then 64 ones cols.
        # PV with this lhsT gives psum rows 0-63 = O_h^T, rows 64-127 = Z
        # replicated 64x (so normalization needs no broadcast).
        vones = [sb.tile([128, 4, 256], F16, tag=f"vones{g}", name=f"vones{g}")
                 for g in range(2)]
        otn = [sb.tile([128, 512], F16, tag=f"otn{nh}", name=f"otn{nh}")
               for nh in range(2)]
        rbc = [[sb.tile([64, 512], F16, tag=f"rbc{nh}{h}",
                        name=f"rbc{nh}{h}") for h in range(2)]
               for nh in range(2)]
        zln = [[sb.tile([64, 512], F16, tag=f"zln{nh}{h}",
                        name=f"zln{nh}{h}") for h in range(2)]
               for nh in range(2)]
        # exp outputs: ACT units write f16 p tiles; DVE units write f32
        # carriers (low halfwords = f16 bits of the fast-exp)
        pa_t = [sb.tile([128, 512], F16, tag=f"pa{i}", name=f"pa{i}")
                for i in range(3)]
        cb_t = [sb.tile([128, 512], F32, tag=f"cb{i}", name=f"cb{i}")
                for i in range(3)]
        # output staging: one contiguous [128, 2, 512] tile per qt pair
        o16_t = [sb.tile([128, 2, C], F16, tag=f"o16{i}", name=f"o16{i}")
                 for i in range(4)]

        # ---- PE warmup scratch: memset on the (idle) vector engine ----
        nc.vector.memset(warm[:], 0.0)

        # ---- input DMAs, one need-ordered sync HWDGE queue: weights
        # first (everything contracts against them), then xc halves
        # (keys), then xp halves (queries).  4-8KB packets throughout ----
        nc.sync.dma_start(out=wall_sb[:], in_=wall)
        nc.sync.dma_start(out=xc_sb[0][:], in_=xc_d[0])
        nc.sync.dma_start(out=xc_sb[1][:], in_=xc_d[1])
        nc.sync.dma_start(out=xp_sb[0][:], in_=xp_d[0])
        nc.sync.dma_start(out=xp_sb[1][:], in_=xp_d[1])

        # ---- constant / zero-pad memsets (overlap the DMA window) ----
        for g in range(2):
            nc.gpsimd.memset(vones[g][:, :, 64:128], 1.0)
            nc.gpsimd.memset(vones[g][:, :, 192:256], 1.0)
        for g in range(2):
            nc.gpsimd.memset(kt_p[0][g][64:128, :], 0.0)
            nc.gpsimd.memset(kt_p[1][g][0:64, :], 0.0)
            nc.gpsimd.memset(qt_p[0][g][64:128, :], 0.0)
            nc.gpsimd.memset(qt_p[1][g][0:64, :], 0.0)

        # ---- psum pools with hand-managed lifetimes (8 banks total).
        # Two independent LIFO sides: left holds {kq} -> {pv1} -> {proj},
        # right holds {warm} -> {v} -> {s3, pv0}; peak usage is exactly 8.
        p3_stack = ExitStack()
        p3 = p3_stack.enter_context(
            tc.tile_pool(name="p3_ps", bufs=1, space="PSUM", side="left"))
        kq = [p3.tile([128, 512], F32, tag=f"kq{g}", name=f"kq{g}")
              for g in range(2)]
        warm_stack = ExitStack()
        wpool = warm_stack.enter_context(
            tc.tile_pool(name="warm_ps", bufs=1, space="PSUM", side="right"))
        warm_ps = wpool.tile([128, 512], F32, tag="warmps")

        # warmup ladder: ramp the PE clock during the DMA window; the
        # tail is fine-grained so KT starts within ~60ns of data arrival
        for i in range(N_WARM_BIG):
            nc.tensor.matmul(out=warm_ps[:], lhsT=warm[:, 0:128],
                             rhs=warm[:], start=True, stop=True)
        for i in range(N_WARM_SMALL):
            nc.tensor.matmul(out=warm_ps[:, 0:128], lhsT=warm[:, 0:128],
                             rhs=warm[:, 0:128], start=True, stop=True)
        warm_stack.close()
        v_stack = ExitStack()
        vpool = v_stack.enter_context(
            tc.tile_pool(name="v_ps", bufs=1, space="PSUM", side="right"))
        v_ps = [vpool.tile([128, 128], F32, tag=f"vps{i}", name=f"vps{i}")
                for i in range(2)]

        def emit_ktv(g):
            # KT: kq[g] = Wk^T Xc_g (accumulate over ct), then evac with
            # one engine per destination tile; V for this key-half
            for ct in range(CT_N):
                nc.tensor.matmul(
                    out=kq[g][:], lhsT=wk_sb[:, ct, :],
                    rhs=xc_sb[g][:, ct, :],
                    start=(ct == 0), stop=(ct == CT_N - 1))
            nc.vector.tensor_copy(out=kt_p[0][g][0:64, :],
                                  in_=kq[g][0:64, :])
            nc.scalar.copy(out=kt_p[1][g][64:128, :],
                           in_=kq[g][64:128, :])
            for kt in range(4 * g, 4 * g + 4):
                vt = v_ps[kt % 2]
                for ct in range(CT_N):
                    nc.tensor.matmul(
                        out=vt[:],
                        lhsT=xc_sb[g][:, ct, (kt % 4) * 128:
                                      (kt % 4) * 128 + 128],
                        rhs=wv_sb[:, ct, :],
                        start=(ct == 0), stop=(ct == CT_N - 1))
                dst = vones[g][:, kt % 4, :].rearrange(
                    "p (q s) -> p q s", q=2)[:, :, 0:64]
                vsrc = vt[:].rearrange("p (q s) -> p q s", q=2)
                nc.vector.tensor_copy(out=dst, in_=vsrc)

        def emit_qt(nh):
            # QT for query-half nh into kq[nh] (banks free after KT evac)
            for ct in range(CT_N):
                nc.tensor.matmul(
                    out=kq[nh][:], lhsT=wq_sb[:, ct, :],
                    rhs=xp_sb[nh][:, ct, :],
                    start=(ct == 0), stop=(ct == CT_N - 1))
            if nh == 0:
                nc.vector.tensor_copy(out=qt_p[0][nh][0:64, :],
                                      in_=kq[nh][0:64, :])
                nc.scalar.copy(out=qt_p[1][nh][64:128, :],
                               in_=kq[nh][64:128, :])
            else:
                nc.scalar.copy(out=qt_p[0][nh][0:64, :],
                               in_=kq[nh][0:64, :])
                nc.vector.tensor_copy(out=qt_p[1][nh][64:128, :],
                                      in_=kq[nh][64:128, :])

        emit_ktv(0)
        emit_ktv(1)
        v_stack.close()
        emit_qt(0)

        # ---- attention loop: query-half-major; per unit one S matmul,
        # one exp (ACT exact / DVE fast alternating), one PV matmul ----
        s_stack = ExitStack()
        s_pool = s_stack.enter_context(
            tc.tile_pool(name="s_ps", bufs=1, space="PSUM", side="right"))
        s4 = [s_pool.tile([128, 512], F32, tag=f"s{j}", name=f"s{j}")
              for j in range(4)]
        pv0_stack = ExitStack()
        pv0_pool = pv0_stack.enter_context(
            tc.tile_pool(name="pv0_ps", bufs=1, space="PSUM", side="right"))
        pv_n = [[pv0_pool.tile([128, 512], F32, tag=f"pv0{h}",
                               name=f"pv0{h}") for h in range(2)], None]

        units = [(kt, h) for kt in range(KT_N) for h in range(2)]
        na, nb = 0, 0
        unit_src = {}

        def emit_st(nh, u):
            kt, h = units[u]
            nc.tensor.matmul(
                out=s4[u % 4][:],
                lhsT=kt_p[h][kt // 4][:, (kt % 4) * 128:(kt % 4) * 128 + 128],
                rhs=qt_p[h][nh][:], start=True, stop=True)

        def emit_exp(nh, u):
            nonlocal na, nb
            s = s4[u % 4]
            if DVE_UNIT[nh * 16 + u]:
                c = cb_t[nb % 3]
                nb += 1
                nc.vector.tensor_scalar(
                    out=c[:], in0=s[:], scalar1=FE_A, scalar2=FE_B,
                    op0=Mult, op1=Add)
                unit_src[u] = c[:].bitcast(F16).rearrange(
                    "p (c two) -> p c two", two=2)[:, :, 0]
            else:
                p = pa_t[na % 3]
                na += 1
                nc.scalar.activation(out=p[:], in_=s[:], func=Exp,
                                     scale=float(SCALE))
                unit_src[u] = p[:]

        def emit_pv(nh, u):
            kt, h = units[u]
            nc.tensor.matmul(
                out=pv_n[nh][h][:],
                lhsT=vones[kt // 4][:, kt % 4, h * 128:(h + 1) * 128],
                rhs=unit_src.pop(u),
                start=(kt == 0), stop=(kt == KT_N - 1))

        def emit_rbc(nh, h):
            # 1/Z = exp(-ln Z) on the replicated Z rows (ln and Exp share
            # an act table set: no reload)
            nc.scalar.activation(out=zln[nh][h][:],
                                 in_=pv_n[nh][h][64:128, :], func=Ln)
            nc.scalar.activation(out=rbc[nh][h][:], in_=zln[nh][h][:],
                                 func=Exp, scale=-1.0)

        def emit_otn(nh, h):
            nc.vector.tensor_mul(out=otn[nh][h * 64:(h + 1) * 64, :],
                                 in0=pv_n[nh][h][0:64, :],
                                 in1=rbc[nh][h][:])

        def emit_proj(nh, qts, out_ps):
            # projection per 128-query tile + staged evac; DMA per pair
            for j, q in enumerate(qts):
                ot = out_ps[j % len(out_ps)]
                nc.tensor.matmul(out=ot[:],
                                 lhsT=otn[nh][:, q * 128:q * 128 + 128],
                                 rhs=wp_sb[:], start=True, stop=True)
                qt = nh * 4 + q
                o16 = o16_t[qt // 2]
                if qt % 2 == 0:
                    nc.vector.tensor_copy(out=o16[:, 0, :], in_=ot[:])
                else:
                    nc.scalar.copy(out=o16[:, 1, :], in_=ot[:])
                    nc.sync.dma_start(out=out[qt // 2], in_=o16[:])

        # --- nh=0 group ---
        for u in range(3):
            emit_st(0, u)
            emit_exp(0, u)
        for u in range(len(units)):
            if u == 2:
                # QT-B slots in once xpB lands; kq[1] free after KT evac
                emit_qt(1)
                p3_stack.close()
            if u + 3 < len(units):
                emit_st(0, u + 3)
                emit_exp(0, u + 3)
            emit_pv(0, u)

        pv1_stack = ExitStack()
        pv1_pool = pv1_stack.enter_context(
            tc.tile_pool(name="pv1_ps", bufs=1, space="PSUM", side="left"))
        pv_n[1] = [pv1_pool.tile([128, 512], F32, tag=f"pv1{h}",
                                 name=f"pv1{h}") for h in range(2)]

        # --- nh=1 group, with the nh=0 normalize/project interleaved ---
        for u in range(3):
            emit_st(1, u)
            emit_exp(1, u)
        proj_stack = ExitStack()
        out_ps = None
        for u in range(len(units)):
            if u == 2:
                emit_rbc(0, 0)
            if u == 4:
                emit_rbc(0, 1)
            if u == 5:
                emit_otn(0, 0)
            if u == 6:
                emit_otn(0, 1)
            if u == 8:
                # pv0 banks free after the otn reads -> projection psum
                pv0_stack.close()
                opool = proj_stack.enter_context(
                    tc.tile_pool(name="proj_ps", bufs=1, space="PSUM",
                                 side="left"))
                out_ps = [opool.tile([128, C], F32, tag=f"ops{i}",
                                     name=f"ops{i}") for i in range(2)]
            if u == 9:
                emit_proj(0, [0, 1], out_ps)
            if u == 11:
                emit_proj(0, [2, 3], out_ps)
            if u + 3 < len(units):
                emit_st(1, u + 3)
                emit_exp(1, u + 3)
            emit_pv(1, u)

        # --- nh=1 tail: keepalives bridge the rbc window, then the
        # normalize -> project -> evac -> DMA chain, finest last ---
        for i in range(7):
            nc.tensor.matmul(out=s4[0][:], lhsT=warm[:, 0:128],
                             rhs=warm[:], start=True, stop=True)
        emit_rbc(1, 0)
        emit_rbc(1, 1)
        emit_otn(1, 0)
        emit_otn(1, 1)
        emit_proj(1, [0, 1], out_ps)
        emit_proj(1, [2, 3], out_ps)
        proj_stack.close()
        pv1_stack.close()
        s_stack.close()


def _get_program():
    global _PROG
    if _PROG is None:
        _PROG = _build_program()
    return _PROG


def _shard_inputs(x_pred, x_ctx, ctx_mask, Wq, Wkv, Wproj):
    """Build the 8 per-core input maps (host-side sharding + packing)."""
    ctx_mask = np.asarray(ctx_mask).astype(bool)
    pidx = np.nonzero(~ctx_mask.reshape(-1))[0]
    cidx = np.nonzero(ctx_mask.reshape(-1))[0]
    pm = [np.where(pidx // T == b)[0] for b in range(B)]
    cm = [np.where(cidx // T == b)[0] for b in range(B)]
    for b in range(B):
        assert len(pm[b]) == T_CTX and len(cm[b]) == T_CTX, (
            "kernel compiled for T_CTX ctx/pred slots per batch row")

    def pack_x(X):  # [SEQ, C] -> two [128, CT_N, 512] halves (4KB rows)
        xt = X.T.astype(np.float16)                 # [C, SEQ]
        full = xt.reshape(CT_N, 128, SEQ).transpose(1, 0, 2)
        return (np.ascontiguousarray(full[:, :, :512]),
                np.ascontiguousarray(full[:, :, 512:]))

    def pack_w(W):  # [C, 128] -> [128, CT_N, 128]
        return np.ascontiguousarray(
            W.astype(np.float16).reshape(CT_N, 128, 128).transpose(1, 0, 2))

    xp_b = [pack_x(x_pred[pm[b]].reshape(SEQ, C)) for b in range(B)]
    xc_b = [pack_x(x_ctx[cm[b]].reshape(SEQ, C)) for b in range(B)]

    wq16 = Wq.astype(np.float16)
    wk16 = Wkv[:, :C].astype(np.float16)
    wv16 = Wkv[:, C:].astype(np.float16)
    wp16 = Wproj.astype(np.float16)

    in_maps = []
    for c in range(NCORE):
        b, hp = divmod(c, 4)
        hc = hp * 128
        wall = np.stack([
            pack_w(wk16[:, hc:hc + 128]),
            pack_w(wq16[:, hc:hc + 128]),
            pack_w(wv16[:, hc:hc + 128]),
            np.ascontiguousarray(
                wp16[hc:hc + 128, :].reshape(128, CT_N, 128)),
        ], axis=1)
        in_maps.append({
            "wall": wall,
            "xcA": xc_b[b][0], "xcB": xc_b[b][1],
            "xpA": xp_b[b][0], "xpB": xp_b[b][1],
        })
    return in_maps, pm


def _unshard_output(results, pm, bproj, dtype):
    full = np.zeros((B * T_CTX, N, C), dtype)
    for b in range(B):
        # out blocks [4, 128, 2, 512]: row q = j*256 + k*128 + p
        acc = results[4 * b]["out"].astype(np.float64)
        for j in range(1, 4):
            acc = acc + results[4 * b + j]["out"]
        acc = acc.transpose(0, 2, 1, 3).reshape(SEQ, C)
        acc = (acc + bproj).astype(dtype)
        full[pm[b]] = acc.reshape(T_CTX, N, C)
    return full


def run(inputs, trace=False, **kwargs):
    """Run the SPMD kernel; returns (full_output, BassKernelResults)."""
    from concourse.bass_utils import run_bass_kernel_spmd

    nc = _get_program()
    in_maps, pm = _shard_inputs(inputs["x_pred"], inputs["x_ctx"],
                                inputs["ctx_mask"], inputs["Wq"],
                                inputs["Wkv"], inputs["Wproj"])
    res = run_bass_kernel_spmd(nc, in_maps, list(range(NCORE)), trace=trace,
                               **kwargs)
    out = _unshard_output(res.results, pm, np.asarray(inputs["bproj"]),
                          np.asarray(inputs["x_pred"]).dtype)
    return out, res


def kernel(x_pred, x_ctx, ctx_mask, Wq, Wkv, Wproj, bproj):
    out, _ = run(dict(x_pred=np.asarray(x_pred), x_ctx=np.asarray(x_ctx),
                      ctx_mask=np.asarray(ctx_mask), Wq=np.asarray(Wq),
                      Wkv=np.asarray(Wkv), Wproj=np.asarray(Wproj),
                      bproj=np.asarray(bproj)))
    return out
